# revision 52
# baseline (speedup 1.0000x reference)
"""Trainium2 Bass kernel for nn_CausalSelfAttention_8443905704568.

Causal self-attention with RoPE + 10-token adapter cross-attention,
B=1, T=2048, C=4096, H=32 heads of hd=128, fp32 I/O.

Strategy: tensor-parallel over heads across 8 NeuronCores (4 heads/core).
Each core computes qkv for its heads (w_attn rows sharded), runs
flash-style attention in transposed orientation (scores^T, so the
probs land partition=key which feeds the P^T @ V matmul directly),
the adapter cross-attention, and a partial output projection
(w_proj columns sharded). Host sums the 8 partial outputs.

Optimizations vs the f32r baseline (898us -> ~660us measured):
- all matmul streams in bf16 (same 1 cyc/row PE rate as f32r, half the
  DMA bytes and SBUF footprint); PSUM/softmax stats stay fp32; final
  out stays fp32.  End-to-end error ~4e-3 rel-max (gate 2e-2).
- q/k/v never round-trip through HBM: they are produced into persistent
  SBUF tiles in phase A and consumed in-place by the attention phase.
- all big operands host-prepped into partition-major [128, ct, cols]
  layouts so every DMA is long-contiguous per partition.
- softmax normalization restructured: the adapter branch is folded into
  the attention PSUM accumulator (pyaT accumulates on top of the pv
  chain after pre-scaling the adapter probs by g*den/aden), so the
  per-block normalize costs ONE [128,512] DVE multiply instead of five.
  The reciprocal rides tiny SBUF<->SBUF reshape DMAs on the scalar
  engine's HWDGE ring (never behind MB-sized transfers) and is deferred
  one block so it never stalls the PE.
- attention i-blocks processed in DESCENDING size order: the dense
  ib=3 blocks keep the PE HAM-warm (K=8/8) across the phase transition
  (HAM oscillation was worth ~2x on matmul issue rate when cold).
- diagonal (causally half-masked) j-tiles compute only the visible
  query range (partial-width scores/exp/mask/pv/den).
- output projection is streamed per attention i-block (interleaved into
  the attention instruction stream) so PE never idles between "phases",
  and the out DMA overlaps compute.
- mask multiplies on DVE only (gpsimd tensor ops have multi-us fixed
  cost and sit on the exp->pv critical path).
- one projection chunk popped after EVERY attention block (PE never
  dips below the HAM activity threshold); both softmax denominators
  leave PSUM in a single 33-partition DVE copy (DVE time is
  free-dim-serial, so it costs the same as one row).
"""

import math
import os

import ml_dtypes
import numpy as np

import concourse.bass as bass
import concourse.mybir as mybir
import concourse.tile as tile
from concourse import bacc
from concourse.bass_utils import run_bass_kernel_spmd

F32 = mybir.dt.float32
F32R = mybir.dt.float32r
BF16 = mybir.dt.bfloat16

T = 2048
C = 4096
NHEAD_TOTAL = 32
NCORES = 8
NH = NHEAD_TOTAL // NCORES      # heads per core = 4
HD = C // NHEAD_TOTAL           # head dim = 128
P = 128
CT = C // P                     # contraction tiles = 32
OW = NH * HD                    # per-pass weight output dim = 512
TB = 256                        # x t-block
NTB = T // TB                   # 8
IB = 512                        # attention i-block
NIB = T // IB                   # 4
NJT = T // P                    # 16 j-tiles
AT = 10                         # adapter tokens
SCALE = 1.0 / math.sqrt(HD)
ASCALE = 1.0 / math.sqrt(C)
EXP = mybir.ActivationFunctionType.Exp
MULT = mybir.AluOpType.mult


def _build_nc():
    nc = bacc.Bacc("TRN2", target_bir_lowering=False, debug=False,
                   num_devices=NCORES)

    # all big operands are host-prepped into partition-major [128, ct, cols]
    # layouts so every DMA is long-contiguous per partition (few descriptors)
    xT_d = nc.dram_tensor("xT", [P, CT, T], BF16, kind="ExternalInput").ap()
    wqkT_d = nc.dram_tensor("wqkT", [P, CT, 2 * OW], BF16,
                            kind="ExternalInput").ap()
    wvT_d = nc.dram_tensor("wvT", [P, CT, OW], BF16,
                           kind="ExternalInput").ap()
    wpT_d = nc.dram_tensor("wpT", [P, NH, C], BF16,
                           kind="ExternalInput").ap()
    awteT_d = nc.dram_tensor("awteT", [P, CT, AT], BF16,
                             kind="ExternalInput").ap()
    cosT_d = nc.dram_tensor("cosT", [P, T], BF16, kind="ExternalInput").ap()
    sinT_d = nc.dram_tensor("sinT", [P, T], BF16, kind="ExternalInput").ap()
    rotT_d = nc.dram_tensor("rotT", [P, P], BF16, kind="ExternalInput").ap()
    masks_d = nc.dram_tensor("masks", [P, NIB, IB], BF16,
                             kind="ExternalInput").ap()
    grow_d = nc.dram_tensor("grow", [1, P], F32, kind="ExternalInput").ap()

    out_d = nc.dram_tensor("out", [T, C], F32, kind="ExternalOutput").ap()

    with tile.TileContext(nc) as tc:
      with tc.tile_pool(name="const", bufs=1) as cpool:
        # persistent qkv / attention-output tiles (never touch HBM)
        qT_sb = cpool.tile([P, NH, T], BF16, name="qT_sb")
        kT_sb = cpool.tile([P, NH, T], BF16, name="kT_sb")
        v_sb = cpool.tile([P, NJT, OW], BF16, name="v_sb")
        yT_sb = cpool.tile([P, NH, T], BF16, name="yT_sb")
        masks_sb = cpool.tile([P, NIB, IB], BF16, name="masks_sb")
        awteT_sb = cpool.tile([P, CT, AT], BF16, name="awteT_sb")
        akT_sb = cpool.tile([P, NH, AT], BF16, name="akT_sb")
        av_sb = cpool.tile([AT, OW], BF16, name="av_sb")
        rotT_sb = cpool.tile([P, P], BF16, name="rotT_sb")
        grow_sb = cpool.tile([1, P], F32, name="grow_sb")
        ones128f = cpool.tile([P, 1], F32, name="ones128f")
        ones1f = cpool.tile([1, P], F32, name="ones1f")
        ones128 = cpool.tile([P, 1], BF16, name="ones128")
        ones1 = cpool.tile([1, P], F32R, name="ones1")

        # masks_sb is deliberately NOT loaded here: it is only needed in
        # phase C, and the first q matmul gates on everything queued ahead
        # of the first x block.
        nc.vector.memset(ones128f[:], 1.0)
        nc.vector.memset(ones1f[:], 1.0)
        with nc.allow_low_precision("exact small integers"):
            nc.vector.tensor_copy(out=ones128[:], in_=ones128f[:])
            nc.vector.tensor_copy(out=ones1[:], in_=ones1f[:])

        # ===== phase A: q pass, then fused k+v pass (x read twice) ========
        with tc.tile_pool(name="cs", bufs=1) as cs_pool, \
             tc.tile_pool(name="wpool", bufs=2) as w_pool, \
             tc.tile_pool(name="xa", bufs=4) as xa_pool, \
             tc.tile_pool(name="ropeA", bufs=2) as rope_pool, \
             tc.tile_pool(name="psA", bufs=4, space="PSUM") as psA, \
             tc.tile_pool(name="psArot", bufs=2, space="PSUM") as psArot, \
             tc.tile_pool(name="psAv", bufs=2, space="PSUM") as psAv:

            cos_sb = cs_pool.tile([P, T], BF16, name="cos_sb")
            sin_sb = cs_pool.tile([P, T], BF16, name="sin_sb")

            # weights rotate through the double-buffered pool:
            # slot0: wq -> wv, slot1: wk.  All DMAs are chunked by
            # ct-quarters so the first matmuls start early; only the first
            # two wq chunks go ahead of the first x block (DMA is FIFO per
            # queue, and the first matmul gates on x + chunk0).
            wq_sb = w_pool.tile([P, CT, OW], BF16, tag="w", name="wq_sb")
            wk_sb = w_pool.tile([P, CT, OW], BF16, tag="w", name="wk_sb")
            CQ = CT // 4

            def load_wq_chunk(c4):
                nc.sync.dma_start(wq_sb[:, bass.ts(c4, CQ), :],
                                  wqkT_d[:, bass.ts(c4, CQ), :OW])

            # first eighth ahead of everything so the first matmul can
            # start as soon as the first x quarter lands
            nc.sync.dma_start(wq_sb[:, 0:CQ // 2, :],
                              wqkT_d[:, 0:CQ // 2, :OW])
            first_x = [None]
            nc.sync.dma_start(rotT_sb[:], rotT_d[:])
            nc.sync.dma_start(awteT_sb[:], awteT_d[:])
            nc.sync.dma_start(grow_sb[:], grow_d[:])

            def rope_tail(pqk, oh, tsl, dst, wide):
                if wide:
                    nc.scalar.copy(akT_sb[:, oh, :], pqk[:, TB:])
                raw = rope_pool.tile([P, TB], BF16, tag="raw", name="raw")
                nc.scalar.copy(raw[:], pqk[:, :TB])
                prot = psArot.tile([P, TB], F32, tag="prot", name="prot")
                nc.tensor.matmul(prot[:], rotT_sb[:], raw[:],
                                 start=True, stop=True)
                t1 = rope_pool.tile([P, TB], F32, tag="t1", name="t1")
                nc.vector.tensor_mul(t1[:], prot[:], sin_sb[:, tsl])
                t2 = rope_pool.tile([P, TB], F32, tag="t2", name="t2")
                nc.vector.tensor_mul(t2[:], pqk[:, :TB], cos_sb[:, tsl])
                nc.vector.tensor_add(dst[:, oh, tsl], t1[:], t2[:])

            def qk_head_block(w_sb, pqk_ncol, oh, xh, tsl, dst, wide):
                pqk = psA.tile([P, TB + AT], F32, tag="pqk", name="pqk")
                for ct in range(CT):
                    nc.tensor.matmul(
                        pqk[:, :pqk_ncol],
                        w_sb[:, ct, oh * HD:(oh + 1) * HD],
                        xh[ct // (CT // 2)][:, ct % (CT // 2), :pqk_ncol],
                        start=(ct == 0), stop=(ct == CT - 1))
                rope_tail(pqk, oh, tsl, dst, wide)

            def qk_block_ct_outer(w_sb, pqk_ncol, xh, tsl, dst, wide):
                # first block of a pass: ct-outer so the matmuls start as
                # soon as the first weight chunk + x half arrive
                pqks = [psA.tile([P, TB + AT], F32, tag="pqk",
                                 name=f"pqk{oh}") for oh in range(NH)]
                for ct in range(CT):
                    for oh in range(NH):
                        nc.tensor.matmul(
                            pqks[oh][:, :pqk_ncol],
                            w_sb[:, ct, oh * HD:(oh + 1) * HD],
                            xh[ct // (CT // 2)][:, ct % (CT // 2), :pqk_ncol],
                            start=(ct == 0), stop=(ct == CT - 1))
                for oh in range(NH):
                    rope_tail(pqks[oh], oh, tsl, dst, wide)

            def load_x_block(tsl, ncol, wide):
                xh = []
                for half in range(2):
                    xa = xa_pool.tile([P, CT // 2, TB + AT], BF16, tag="xa",
                                      name="xa")
                    csl = bass.ts(half, CT // 2)
                    if first_x[0] is None:
                        # split the very first x half so ct 0-7's matmuls
                        # start after ~1.25MB instead of the full pile
                        first_x[0] = True
                        nc.sync.dma_start(xa[:, :CT // 4, :TB],
                                          xT_d[:, 0:CT // 4, tsl])
                        nc.sync.dma_start(xa[:, CT // 4:, :TB],
                                          xT_d[:, CT // 4:CT // 2, tsl])
                    else:
                        nc.sync.dma_start(xa[:, :, :TB], xT_d[:, csl, tsl])
                    if wide:
                        nc.sync.dma_start(xa[:, :, TB:], awteT_d[:, csl, :])
                    xh.append(xa)
                return xh

            # ---- q pass ----
            # wk/wv chunk DMAs ride between the x blocks so they never
            # sit ahead of x in the queue
            wv_sb = w_pool.tile([P, CT, OW], BF16, tag="w", name="wv_sb")
            for tb in range(NTB):
                tsl = bass.ts(tb, TB)
                xh = load_x_block(tsl, TB, False)
                if tb == 0:
                    # rest of chunk 0 + chunks 1-3 must be emitted BEFORE
                    # tb0's compute: the ct-outer block reads all 32 ct tiles
                    nc.sync.dma_start(wq_sb[:, CQ // 2:CQ, :],
                                      wqkT_d[:, CQ // 2:CQ, :OW])
                    load_wq_chunk(1)
                    load_wq_chunk(2)
                    load_wq_chunk(3)
                    nc.sync.dma_start(cos_sb[:], cosT_d[:])
                    nc.sync.dma_start(sin_sb[:], sinT_d[:])
                elif tb == 1:
                    pass
                elif tb < 6:
                    c4 = tb - 2
                    nc.sync.dma_start(wk_sb[:, bass.ts(c4, CQ), :],
                                      wqkT_d[:, bass.ts(c4, CQ), OW:])
                else:
                    for c4 in (2 * (tb - 6), 2 * (tb - 6) + 1):
                        nc.sync.dma_start(wv_sb[:, bass.ts(c4, CQ), :],
                                          wvT_d[:, bass.ts(c4, CQ), :])
                if tb == 0:
                    qk_block_ct_outer(wq_sb, TB, xh, tsl, qT_sb, False)
                else:
                    for oh in range(NH):
                        qk_head_block(wq_sb, TB, oh, xh, tsl, qT_sb, False)

            # ---- fused k+v pass (wv loaded during the q pass) ----
            nc.sync.dma_start(masks_sb[:], masks_d[:])
            for tb in range(NTB):
                tsl = bass.ts(tb, TB)
                wide = (tb == 0)
                ncol = TB + AT if wide else TB
                xh = load_x_block(tsl, ncol, wide)
                if tb == 0:
                    qk_block_ct_outer(wk_sb, ncol, xh, tsl, kT_sb, wide)
                else:
                    for oh in range(NH):
                        qk_head_block(wk_sb, ncol, oh, xh, tsl, kT_sb, wide)
                for sub in range(TB // P):
                    tt = tb * (TB // P) + sub
                    pv = psAv.tile([P, OW], F32, tag="pv", name="pv")
                    for ct in range(CT):
                        nc.tensor.matmul(
                            pv[:],
                            xh[ct // (CT // 2)][:, ct % (CT // 2),
                                                sub * P:(sub + 1) * P],
                            wv_sb[:, ct, :],
                            start=(ct == 0), stop=(ct == CT - 1))
                    nc.scalar.copy(v_sb[:, tt, :], pv[:])

            # adapter-v at the end of the fused pass
            pav = psAv.tile([AT, OW], F32, tag="pv", name="pav")
            for ct in range(CT):
                nc.tensor.matmul(pav[:], awteT_sb[:, ct, :], wv_sb[:, ct, :],
                                 start=(ct == 0), stop=(ct == CT - 1))
            nc.scalar.copy(av_sb[:], pav[:])

        # ========== phase C: attention + streamed output projection =======
        with tc.tile_pool(name="wp2", bufs=1) as wp2_pool, \
             tc.tile_pool(name="expp", bufs=6) as exp_pool, \
             tc.tile_pool(name="small", bufs=3) as small_pool, \
             tc.tile_pool(name="outp", bufs=2) as out_pool, \
             tc.tile_pool(name="psS", bufs=3, space="PSUM") as psS, \
             tc.tile_pool(name="psY", bufs=2, space="PSUM") as psY, \
             tc.tile_pool(name="psSm", bufs=1, space="PSUM") as psSm, \
             tc.tile_pool(name="psPo", bufs=2, space="PSUM") as psPo:

            wp_sb = wp2_pool.tile([P, NH, C], BF16, name="wp_sb")
            for hh in range(NH):
                nc.sync.dma_start(wp_sb[:, hh, :], wpT_d[:, hh, :])

            # combines are deferred TWO blocks so the reciprocal round trip
            # (DVE copy -> reshape DMA -> recip -> reshape DMA) never stalls
            # the PE, even across the small ib=0 blocks
            pending = []

            def emit_r(prev):
                # r = g * den / aden, computed one block ahead of the
                # combine so its PE ops never wait on the vector engine
                (h, ib, pyT, ea, dsb_a, rec_a, rec_b) = prev
                r_sb = small_pool.tile([1, IB], F32R, tag="r_sb", name="r_sb")
                with nc.allow_low_precision("f32r is full-width fp32"):
                    nc.vector.scalar_tensor_tensor(
                        r_sb[:], dsb_a[:], grow_sb[0:1, 0:1], rec_b[:],
                        MULT, MULT)
                return r_sb

            def emit_combine(prev, r_sb):
                (h, ib, pyT, ea, dsb_a, rec_a, rec_b) = prev
                # broadcast r over the 10 adapter rows, pre-scale the
                # adapter probs, and fold the adapter output into the
                # attention accumulator (same PSUM bank)
                pea10 = psPo.tile([AT, IB], F32, tag="po", name="pea10")
                nc.tensor.matmul(pea10[:], ones1[:, :AT], r_sb[:],
                                 start=True, stop=True)
                ea_s = small_pool.tile([AT, IB], BF16, tag="ea_s",
                                       name="ea_s")
                with nc.allow_low_precision("bf16 probs"):
                    nc.vector.tensor_mul(ea_s[:], ea[:], pea10[:])
                nc.tensor.matmul(pyT[:], av_sb[:, h * HD:(h + 1) * HD],
                                 ea_s[:], start=False, stop=True,
                                 skip_group_check=True)
                # broadcast 1/den and apply in ONE [128,512] multiply
                b1 = psPo.tile([P, IB], F32, tag="po", name="b1")
                nc.tensor.matmul(b1[:], ones1[:], rec_a[:],
                                 start=True, stop=True)
                b1s = small_pool.tile([P, IB], F32, tag="b1s", name="b1s")
                nc.scalar.copy(b1s[:], b1[:])
                with nc.allow_low_precision("bf16 attention output"):
                    nc.vector.tensor_mul(yT_sb[:, h, bass.ts(ib, IB)],
                                         pyT[:], b1s[:])

            def emit_proj_chunk(gt, act_obs=4):
                # output projection for one 128-token tile (all heads);
                # act_obs of the 8 PSUM evacuations go to ACT, rest to DVE
                osb = out_pool.tile([P, C], F32, tag="osb", name="osb")
                for ob in range(C // 512):
                    po = psPo.tile([P, 512], F32, tag="po", name="po")
                    for hh in range(NH):
                        nc.tensor.matmul(
                            po[:],
                            yT_sb[:, hh, bass.ts(gt, P)],
                            wp_sb[:, hh, bass.ts(ob, 512)],
                            start=(hh == 0), stop=(hh == NH - 1))
                    if ob < act_obs:
                        nc.scalar.copy(osb[:, bass.ts(ob, 512)], po[:])
                    else:
                        nc.vector.tensor_copy(
                            out=osb[:, bass.ts(ob, 512)], in_=po[:])
                    if ob == 3:
                        nc.sync.dma_start(out_d[bass.ts(gt, P), :C // 2],
                                          osb[:, :C // 2])
                nc.sync.dma_start(out_d[bass.ts(gt, P), C // 2:],
                                  osb[:, C // 2:])

            proj_queue = []

            # block order (2,1,0,3): medium blocks first keep the PE
            # HAM-warm across the A->C transition; the dense ib=3 blocks and
            # their projection tiles form the tail, so the end of the kernel
            # is pure back-to-back matmul instead of overhead-dominated
            # small blocks
            IB_ORDER = (3, 2, 1, 0)
            for ibx, ib in enumerate(IB_ORDER):
                isl = bass.ts(ib, IB)
                nj = 4 * ib + 4
                if ibx >= 1:
                    prev_ib = IB_ORDER[ibx - 1]
                    proj_queue.extend(4 * prev_ib + tt for tt in range(4))
                for h in range(NH):
                    qT_h = qT_sb[:, h, :]

                    # adapter scores first: exp(ea) runs during the jt loop
                    pasT = psPo.tile([AT, IB], F32, tag="po", name="pasT")
                    nc.tensor.matmul(pasT[:], akT_sb[:, h, :], qT_h[:, isl],
                                     start=True, stop=True)
                    ea = small_pool.tile([AT, IB], BF16, tag="ea", name="ea")
                    nc.scalar.activation(ea[:], pasT[:], EXP, scale=ASCALE)

                    # emit r for the newest pending block (one block ahead
                    # of its combine)
                    if pending and pending[-1][1] is None:
                        pending[-1][1] = emit_r(pending[-1][0])

                    pyT = psY.tile([P, IB], F32, tag="py", name="pyT")
                    denp = psSm.tile([33, IB], F32, tag="sm", name="denp")

                    eTs = {}

                    def emit_scores(jt, h=h, ib=ib, isl=isl, qT_h=qT_h,
                                    eTs=eTs):
                        # diagonal tiles only see queries i >= 128*d within
                        # the block: compute the narrowed column range only
                        d = jt - 4 * ib
                        off = max(0, d) * P
                        qsl = slice(ib * IB + off, (ib + 1) * IB)
                        psT = psS.tile([P, IB], F32, tag="ps", name="psT")
                        nc.tensor.matmul(
                            psT[:, off:], kT_sb[:, h, bass.ts(jt, P)],
                            qT_h[:, qsl], start=True, stop=True)
                        eT = exp_pool.tile([P, IB], BF16, tag="eT",
                                           name="eT")
                        nc.scalar.activation(eT[:, off:], psT[:, off:], EXP,
                                             scale=SCALE)
                        if d >= 0:
                            # masks always on DVE: gpsimd tensor ops have
                            # multi-us fixed cost and sit on the eT->pv
                            # critical path
                            with nc.allow_low_precision("bf16 probs"):
                                nc.vector.tensor_mul(
                                    eT[:, off:], eT[:, off:],
                                    masks_sb[:, d, off:])
                        eTs[jt] = (eT, off)

                    emit_scores(0)
                    emit_scores(1)
                    emit_scores(2)
                    for jt in range(nj):
                        if jt + 3 < nj:
                            emit_scores(jt + 3)
                        if jt == 1:
                            # adapter denominator at partition 32 of the den
                            # bank; emitted a few matmuls into the block so
                            # the PE never waits on the ea exp chain
                            nc.tensor.matmul(denp[32:33, :],
                                             ones128[:AT, :], ea[:],
                                             start=True, stop=True,
                                             tile_position=(0, 32))
                        if jt == nj - 1 and pending:
                            prev, prev_r = pending.pop(0)
                            emit_combine(prev, prev_r)
                        eT, off = eTs.pop(jt)
                        nc.tensor.matmul(pyT[:, off:],
                                         v_sb[:, jt, h * HD:(h + 1) * HD],
                                         eT[:, off:], start=(jt == 0),
                                         stop=False, skip_group_check=True)
                        nc.tensor.matmul(denp[0:1, off:], ones128[:],
                                         eT[:, off:], start=(jt == 0),
                                         stop=(jt == nj - 1),
                                         skip_group_check=True)

                    # denominators -> SBUF, reshape onto all 128 lanes via
                    # tiny SBUF<->SBUF DMAs, reciprocal, reshape back.  The
                    # DMAs ride the scalar engine's HWDGE ring so they never
                    # queue behind the megabyte-sized wp/out transfers on
                    # the sync ring.
                    # one 33-partition copy moves BOTH denominators out of
                    # PSUM: DVE time is free-dim-serial, so this costs the
                    # same as a single [1,512] copy
                    dsb = small_pool.tile([33, IB], F32, tag="dsb",
                                          name="dsb")
                    nc.vector.tensor_copy(out=dsb[:], in_=denp[:])
                    dsb_a = dsb[0:1, :]
                    dsb_b = dsb[32:33, :]
                    rsh = small_pool.tile([P, 2, IB // P], F32, tag="rsh",
                                          name="rsh")
                    nc.scalar.dma_start(
                        rsh[:, 0, :], dsb_a.rearrange("x (p e) -> x p e", p=P))
                    nc.scalar.dma_start(
                        rsh[:, 1, :], dsb_b.rearrange("x (p e) -> x p e", p=P))
                    rrec = small_pool.tile([P, 2, IB // P], F32R, tag="rrec",
                                           name="rrec")
                    with nc.allow_low_precision(
                            "f32r is full-width fp32 storage"):
                        nc.vector.reciprocal(rrec[:], rsh[:])
                    rec_a = small_pool.tile([1, IB], F32R, tag="rec_a",
                                            name="rec_a")
                    nc.scalar.dma_start(
                        rec_a.rearrange("x (p e) -> x p e", p=P),
                        rrec[:, 0, :])
                    rec_b = small_pool.tile([1, IB], F32R, tag="rec_b",
                                            name="rec_b")
                    nc.scalar.dma_start(
                        rec_b.rearrange("x (p e) -> x p e", p=P),
                        rrec[:, 1, :])
                    pending.append([(h, ib, pyT, ea, dsb_a, rec_a, rec_b),
                                    None])

                    # proj tiles for i-block ib-1 become legal only once
                    # combine(ib-1, h3) has been emitted, i.e. during block
                    # (ib, h1) — so pop 1 chunk at h1/h2 and 2 at h3
                    if ib == IB_ORDER[-1]:
                        # keep one chunk in reserve: it provides PE cover
                        # for the final combine's reciprocal latency
                        npop = 0 if h == NH - 1 else 1
                    else:
                        npop = 1
                    for _ in range(npop):
                        if proj_queue:
                            # exp load on ACT scales with nj; shift the PSUM
                            # evacuations toward DVE in the dense blocks
                            emit_proj_chunk(proj_queue.pop(0),
                                            act_obs={4: 4, 8: 3, 12: 2, 16: 2}[nj])

            # tail: reserved chunk first (PE cover for the last combine's
            # reciprocal round trip), then the final combine + last tiles
            if proj_queue:
                emit_proj_chunk(proj_queue.pop(0))
            for prev, prev_r in pending:
                if prev_r is None:
                    prev_r = emit_r(prev)
                emit_combine(prev, prev_r)
            proj_queue.extend(4 * IB_ORDER[-1] + tt for tt in range(4))
            for gt in proj_queue:
                emit_proj_chunk(gt)

    nc.compile()
    return nc


LAST_RESULT = None

_ROPE_CACHE = None


def _rope_cos_sin_T():
    global _ROPE_CACHE
    if _ROPE_CACHE is None:
        bf = ml_dtypes.bfloat16
        theta = 1.0 / (10000.0 ** (np.arange(0, HD, 2, dtype=np.float32) / HD))
        idx = np.outer(np.arange(T, dtype=np.float32), theta)  # [T, 64]
        full = np.concatenate([idx, idx], axis=1)              # [T, 128]
        _ROPE_CACHE = (np.ascontiguousarray(np.cos(full).T).astype(bf),
                       np.ascontiguousarray(np.sin(full).T).astype(bf))
    return _ROPE_CACHE


def kernel(x, w_attn, w_proj, adapter_wte, gating):
    bf = ml_dtypes.bfloat16
    x = np.asarray(x, np.float32)
    w_attn = np.asarray(w_attn, np.float32)
    w_proj = np.asarray(w_proj, np.float32)
    adapter_wte = np.asarray(adapter_wte, np.float32)
    gating = np.asarray(gating, np.float32)

    def pmajor(a_cT, nrow):
        # [C, cols] -> [128, C//128, cols] partition-major contiguous
        return np.ascontiguousarray(
            a_cT.reshape(nrow, P, -1).transpose(1, 0, 2)).astype(bf)

    xT = pmajor(x[0].T, CT)                      # [128, 32, T]
    awteT = pmajor(adapter_wte.T, CT)            # [128, 32, 10]
    cosT, sinT = _rope_cos_sin_T()

    # rotate-half as a matmul: rot = R @ q (in [d, t] layout); pass R^T
    R = np.zeros((P, P), np.float32)
    for d in range(64):
        R[d, d + 64] = -1.0
        R[d + 64, d] = 1.0
    rotT = np.ascontiguousarray(R.T).astype(bf)

    # causal masks for the 4 diagonal alignments of a [128j, 512i] block
    pp = np.arange(P)[:, None]
    ff = np.arange(IB)[None, :]
    masks = np.stack([(pp + P * k <= ff) for k in range(NIB)],
                     axis=1).astype(bf)                     # [128, 4, 512]
    masks = np.ascontiguousarray(masks)

    grow = np.full((1, P), float(gating[0]), np.float32)

    nc = _build_nc()

    in_maps = []
    for m in range(NCORES):
        wq = w_attn[OW * m: OW * (m + 1)]
        wk = w_attn[C + OW * m: C + OW * (m + 1)]
        wv = w_attn[2 * C + OW * m: 2 * C + OW * (m + 1)]
        wqkT = pmajor(np.concatenate([wq, wk], axis=0).T, CT)
        wvT = pmajor(wv.T, CT)
        wpT = pmajor(w_proj[:, OW * m: OW * (m + 1)].T, NH)
        in_maps.append({
            "xT": xT, "wqkT": wqkT, "wvT": wvT, "wpT": wpT,
            "awteT": awteT, "cosT": cosT, "sinT": sinT, "rotT": rotT,
            "masks": masks, "grow": grow,
        })

    trace = bool(int(os.environ.get("BASS_KERNEL_TRACE", "0")))
    res = run_bass_kernel_spmd(nc, in_maps, core_ids=list(range(NCORES)),
                               trace=trace)
    global LAST_RESULT
    LAST_RESULT = res
    if trace:
        print("HW exec time:", res.exec_time_ns, "ns")
        print("trace:", res.instructions_and_trace[1]
              if res.instructions_and_trace else None)

    out = np.zeros((T, C), np.float64)
    for r in res.results:
        out += r["out"].astype(np.float64)
    return out.astype(np.float32)[None]


# revision 53
# speedup vs baseline: 1.1037x; 1.1037x over previous
"""Trainium2 Bass kernel for nn_CausalSelfAttention_8443905704568.

Causal self-attention with RoPE + 10-token adapter cross-attention,
B=1, T=2048, C=4096, H=32 heads of hd=128, fp32 I/O.

Strategy: tensor-parallel over heads across 8 NeuronCores (4 heads/core).
Each core computes qkv for its heads (w_attn rows sharded), runs
flash-style attention in transposed orientation (scores^T, so the
probs land partition=key which feeds the P^T @ V matmul directly),
the adapter cross-attention, and a partial output projection
(w_proj columns sharded). Host sums the 8 partial outputs.

Optimizations vs the f32r baseline (898us -> ~660us measured):
- all matmul streams in bf16 (same 1 cyc/row PE rate as f32r, half the
  DMA bytes and SBUF footprint); PSUM/softmax stats stay fp32; final
  out stays fp32.  End-to-end error ~4e-3 rel-max (gate 2e-2).
- q/k/v never round-trip through HBM: they are produced into persistent
  SBUF tiles in phase A and consumed in-place by the attention phase.
- all big operands host-prepped into partition-major [128, ct, cols]
  layouts so every DMA is long-contiguous per partition.
- softmax normalization restructured: the adapter branch is folded into
  the attention PSUM accumulator (pyaT accumulates on top of the pv
  chain after pre-scaling the adapter probs by g*den/aden), so the
  per-block normalize costs ONE [128,512] DVE multiply instead of five.
  The reciprocal rides tiny SBUF<->SBUF reshape DMAs on the scalar
  engine's HWDGE ring (never behind MB-sized transfers) and is deferred
  one block so it never stalls the PE.
- attention i-blocks processed in DESCENDING size order: the dense
  ib=3 blocks keep the PE HAM-warm (K=8/8) across the phase transition
  (HAM oscillation was worth ~2x on matmul issue rate when cold).
- diagonal (causally half-masked) j-tiles compute only the visible
  query range (partial-width scores/exp/mask/pv/den).
- output projection is streamed per attention i-block (interleaved into
  the attention instruction stream) so PE never idles between "phases",
  and the out DMA overlaps compute.
- mask multiplies on DVE only (gpsimd tensor ops have multi-us fixed
  cost and sit on the exp->pv critical path).
- one projection chunk popped after EVERY attention block (PE never
  dips below the HAM activity threshold); both softmax denominators
  leave PSUM in a single 33-partition DVE copy (DVE time is
  free-dim-serial, so it costs the same as one row).
"""

import math
import os

import ml_dtypes
import numpy as np

import concourse.bass as bass
import concourse.mybir as mybir
import concourse.tile as tile
from concourse import bacc
from concourse.bass_utils import run_bass_kernel_spmd

F32 = mybir.dt.float32
F32R = mybir.dt.float32r
BF16 = mybir.dt.bfloat16

T = 2048
C = 4096
NHEAD_TOTAL = 32
NCORES = 8
NH = NHEAD_TOTAL // NCORES      # heads per core = 4
HD = C // NHEAD_TOTAL           # head dim = 128
P = 128
CT = C // P                     # contraction tiles = 32
OW = NH * HD                    # per-pass weight output dim = 512
TB = 256                        # x t-block
NTB = T // TB                   # 8
IB = 512                        # attention i-block
NIB = T // IB                   # 4
NJT = T // P                    # 16 j-tiles
AT = 10                         # adapter tokens
SCALE = 1.0 / math.sqrt(HD)
ASCALE = 1.0 / math.sqrt(C)
EXP = mybir.ActivationFunctionType.Exp
MULT = mybir.AluOpType.mult


def _build_nc():
    nc = bacc.Bacc("TRN2", target_bir_lowering=False, debug=False,
                   num_devices=NCORES)

    # all big operands are host-prepped into partition-major [128, ct, cols]
    # layouts so every DMA is long-contiguous per partition (few descriptors)
    xT_d = nc.dram_tensor("xT", [P, CT, T], BF16, kind="ExternalInput").ap()
    wqkT_d = nc.dram_tensor("wqkT", [P, CT, 2 * OW], BF16,
                            kind="ExternalInput").ap()
    wvT_d = nc.dram_tensor("wvT", [P, CT, OW], BF16,
                           kind="ExternalInput").ap()
    wpT_d = nc.dram_tensor("wpT", [P, NH, C], BF16,
                           kind="ExternalInput").ap()
    awteT_d = nc.dram_tensor("awteT", [P, CT, AT], BF16,
                             kind="ExternalInput").ap()
    cosT_d = nc.dram_tensor("cosT", [P, T], BF16, kind="ExternalInput").ap()
    sinT_d = nc.dram_tensor("sinT", [P, T], BF16, kind="ExternalInput").ap()
    rotT_d = nc.dram_tensor("rotT", [P, P], BF16, kind="ExternalInput").ap()
    masks_d = nc.dram_tensor("masks", [P, NIB, IB], BF16,
                             kind="ExternalInput").ap()
    grow_d = nc.dram_tensor("grow", [1, P], F32, kind="ExternalInput").ap()

    out_d = nc.dram_tensor("out", [T, C], F32, kind="ExternalOutput").ap()

    with tile.TileContext(nc) as tc:
      with tc.tile_pool(name="const", bufs=1) as cpool:
        # persistent qkv / attention-output tiles (never touch HBM)
        qT_sb = cpool.tile([P, NH, T], BF16, name="qT_sb")
        kT_sb = cpool.tile([P, NH, T], BF16, name="kT_sb")
        v_sb = cpool.tile([P, NJT, OW], BF16, name="v_sb")
        yT_sb = cpool.tile([P, NH, T], BF16, name="yT_sb")
        masks_sb = cpool.tile([P, NIB, IB], BF16, name="masks_sb")
        awteT_sb = cpool.tile([P, CT, AT], BF16, name="awteT_sb")
        akT_sb = cpool.tile([P, NH, AT], BF16, name="akT_sb")
        av_sb = cpool.tile([AT, OW], BF16, name="av_sb")
        rotT_sb = cpool.tile([P, P], BF16, name="rotT_sb")
        grow_sb = cpool.tile([1, P], F32, name="grow_sb")
        ones128f = cpool.tile([P, 1], F32, name="ones128f")
        ones1f = cpool.tile([1, P], F32, name="ones1f")
        ones128 = cpool.tile([P, 1], BF16, name="ones128")
        ones1 = cpool.tile([1, P], F32R, name="ones1")

        # masks_sb is deliberately NOT loaded here: it is only needed in
        # phase C, and the first q matmul gates on everything queued ahead
        # of the first x block.
        nc.vector.memset(ones128f[:], 1.0)
        nc.vector.memset(ones1f[:], 1.0)
        with nc.allow_low_precision("exact small integers"):
            nc.vector.tensor_copy(out=ones128[:], in_=ones128f[:])
            nc.vector.tensor_copy(out=ones1[:], in_=ones1f[:])

        # ===== phase A: q pass, then fused k+v pass (x read twice) ========
        with tc.tile_pool(name="cs", bufs=1) as cs_pool, \
             tc.tile_pool(name="wpool", bufs=2) as w_pool, \
             tc.tile_pool(name="xa", bufs=4) as xa_pool, \
             tc.tile_pool(name="ropeA", bufs=2) as rope_pool, \
             tc.tile_pool(name="psA", bufs=4, space="PSUM") as psA, \
             tc.tile_pool(name="psArot", bufs=2, space="PSUM") as psArot, \
             tc.tile_pool(name="psAv", bufs=2, space="PSUM") as psAv:

            cos_sb = cs_pool.tile([P, T], BF16, name="cos_sb")
            sin_sb = cs_pool.tile([P, T], BF16, name="sin_sb")

            # weights rotate through the double-buffered pool:
            # slot0: wq -> wv, slot1: wk.  All DMAs are chunked by
            # ct-quarters so the first matmuls start early; only the first
            # two wq chunks go ahead of the first x block (DMA is FIFO per
            # queue, and the first matmul gates on x + chunk0).
            wq_sb = w_pool.tile([P, CT, OW], BF16, tag="w", name="wq_sb")
            wk_sb = w_pool.tile([P, CT, OW], BF16, tag="w", name="wk_sb")
            CQ = CT // 4

            def load_wq_chunk(c4):
                nc.sync.dma_start(wq_sb[:, bass.ts(c4, CQ), :],
                                  wqkT_d[:, bass.ts(c4, CQ), :OW])

            # first eighth ahead of everything so the first matmul can
            # start as soon as the first x quarter lands
            nc.sync.dma_start(wq_sb[:, 0:CQ // 2, :],
                              wqkT_d[:, 0:CQ // 2, :OW])
            first_x = [None]
            nc.sync.dma_start(rotT_sb[:], rotT_d[:])
            nc.sync.dma_start(awteT_sb[:], awteT_d[:])
            nc.sync.dma_start(grow_sb[:], grow_d[:])

            def rope_tail(pqk, oh, tsl, dst, wide):
                if wide:
                    nc.scalar.copy(akT_sb[:, oh, :], pqk[:, TB:])
                raw = rope_pool.tile([P, TB], BF16, tag="raw", name="raw")
                nc.scalar.copy(raw[:], pqk[:, :TB])
                prot = psArot.tile([P, TB], F32, tag="prot", name="prot")
                nc.tensor.matmul(prot[:], rotT_sb[:], raw[:],
                                 start=True, stop=True)
                t1 = rope_pool.tile([P, TB], F32, tag="t1", name="t1")
                nc.vector.tensor_mul(t1[:], prot[:], sin_sb[:, tsl])
                t2 = rope_pool.tile([P, TB], F32, tag="t2", name="t2")
                nc.vector.tensor_mul(t2[:], pqk[:, :TB], cos_sb[:, tsl])
                nc.vector.tensor_add(dst[:, oh, tsl], t1[:], t2[:])

            def qk_head_block(w_sb, pqk_ncol, oh, xh, tsl, dst, wide):
                pqk = psA.tile([P, TB + AT], F32, tag="pqk", name="pqk")
                for ct in range(CT):
                    nc.tensor.matmul(
                        pqk[:, :pqk_ncol],
                        w_sb[:, ct, oh * HD:(oh + 1) * HD],
                        xh[ct // (CT // 2)][:, ct % (CT // 2), :pqk_ncol],
                        start=(ct == 0), stop=(ct == CT - 1))
                rope_tail(pqk, oh, tsl, dst, wide)

            def qk_block_ct_outer(w_sb, pqk_ncol, xh, tsl, dst, wide):
                # first block of a pass: ct-outer so the matmuls start as
                # soon as the first weight chunk + x half arrive
                pqks = [psA.tile([P, TB + AT], F32, tag="pqk",
                                 name=f"pqk{oh}") for oh in range(NH)]
                for ct in range(CT):
                    for oh in range(NH):
                        nc.tensor.matmul(
                            pqks[oh][:, :pqk_ncol],
                            w_sb[:, ct, oh * HD:(oh + 1) * HD],
                            xh[ct // (CT // 2)][:, ct % (CT // 2), :pqk_ncol],
                            start=(ct == 0), stop=(ct == CT - 1))
                for oh in range(NH):
                    rope_tail(pqks[oh], oh, tsl, dst, wide)

            def load_x_block(tsl, ncol, wide):
                xh = []
                for half in range(2):
                    xa = xa_pool.tile([P, CT // 2, TB + AT], BF16, tag="xa",
                                      name="xa")
                    csl = bass.ts(half, CT // 2)
                    if first_x[0] is None:
                        # split the very first x half so ct 0-7's matmuls
                        # start after ~1.25MB instead of the full pile
                        first_x[0] = True
                        nc.sync.dma_start(xa[:, :CT // 4, :TB],
                                          xT_d[:, 0:CT // 4, tsl])
                        nc.sync.dma_start(xa[:, CT // 4:, :TB],
                                          xT_d[:, CT // 4:CT // 2, tsl])
                    else:
                        nc.sync.dma_start(xa[:, :, :TB], xT_d[:, csl, tsl])
                    if wide:
                        nc.sync.dma_start(xa[:, :, TB:], awteT_d[:, csl, :])
                    xh.append(xa)
                return xh

            # ---- q pass ----
            # wk/wv chunk DMAs ride between the x blocks so they never
            # sit ahead of x in the queue
            wv_sb = w_pool.tile([P, CT, OW], BF16, tag="w", name="wv_sb")
            for tb in range(NTB):
                tsl = bass.ts(tb, TB)
                xh = load_x_block(tsl, TB, False)
                if tb == 0:
                    # rest of chunk 0 + chunks 1-3 must be emitted BEFORE
                    # tb0's compute: the ct-outer block reads all 32 ct tiles
                    nc.sync.dma_start(wq_sb[:, CQ // 2:CQ, :],
                                      wqkT_d[:, CQ // 2:CQ, :OW])
                    load_wq_chunk(1)
                    load_wq_chunk(2)
                    load_wq_chunk(3)
                    nc.sync.dma_start(cos_sb[:], cosT_d[:])
                    nc.sync.dma_start(sin_sb[:], sinT_d[:])
                elif tb == 1:
                    pass
                elif tb < 6:
                    c4 = tb - 2
                    nc.sync.dma_start(wk_sb[:, bass.ts(c4, CQ), :],
                                      wqkT_d[:, bass.ts(c4, CQ), OW:])
                else:
                    for c4 in (2 * (tb - 6), 2 * (tb - 6) + 1):
                        nc.sync.dma_start(wv_sb[:, bass.ts(c4, CQ), :],
                                          wvT_d[:, bass.ts(c4, CQ), :])
                if tb == 0:
                    qk_block_ct_outer(wq_sb, TB, xh, tsl, qT_sb, False)
                else:
                    for oh in range(NH):
                        qk_head_block(wq_sb, TB, oh, xh, tsl, qT_sb, False)

            # ---- fused k+v pass (wv loaded during the q pass) ----
            nc.sync.dma_start(masks_sb[:], masks_d[:])
            for tb in range(NTB):
                tsl = bass.ts(tb, TB)
                wide = (tb == 0)
                ncol = TB + AT if wide else TB
                xh = load_x_block(tsl, ncol, wide)
                if tb == 0:
                    qk_block_ct_outer(wk_sb, ncol, xh, tsl, kT_sb, wide)
                else:
                    for oh in range(NH):
                        qk_head_block(wk_sb, ncol, oh, xh, tsl, kT_sb, wide)
                for sub in range(TB // P):
                    tt = tb * (TB // P) + sub
                    pv = psAv.tile([P, OW], F32, tag="pv", name="pv")
                    for ct in range(CT):
                        nc.tensor.matmul(
                            pv[:],
                            xh[ct // (CT // 2)][:, ct % (CT // 2),
                                                sub * P:(sub + 1) * P],
                            wv_sb[:, ct, :],
                            start=(ct == 0), stop=(ct == CT - 1))
                    nc.scalar.copy(v_sb[:, tt, :], pv[:])

            # adapter-v at the end of the fused pass
            pav = psAv.tile([AT, OW], F32, tag="pv", name="pav")
            for ct in range(CT):
                nc.tensor.matmul(pav[:], awteT_sb[:, ct, :], wv_sb[:, ct, :],
                                 start=(ct == 0), stop=(ct == CT - 1))
            nc.scalar.copy(av_sb[:], pav[:])

        # ========== phase C: attention + streamed output projection =======
        with tc.tile_pool(name="wp2", bufs=1) as wp2_pool, \
             tc.tile_pool(name="expp", bufs=6) as exp_pool, \
             tc.tile_pool(name="small", bufs=3) as small_pool, \
             tc.tile_pool(name="outp", bufs=2) as out_pool, \
             tc.tile_pool(name="psS", bufs=3, space="PSUM") as psS, \
             tc.tile_pool(name="psY", bufs=2, space="PSUM") as psY, \
             tc.tile_pool(name="psSm", bufs=1, space="PSUM") as psSm, \
             tc.tile_pool(name="psPo", bufs=2, space="PSUM") as psPo:

            wp_sb = wp2_pool.tile([P, NH, C], BF16, name="wp_sb")
            for hh in range(NH):
                nc.sync.dma_start(wp_sb[:, hh, :], wpT_d[:, hh, :])

            # combines are deferred TWO blocks so the reciprocal round trip
            # (DVE copy -> reshape DMA -> recip -> reshape DMA) never stalls
            # the PE, even across the small ib=0 blocks
            pending = []

            def emit_r(prev):
                # r = g * den / aden, computed one block ahead of the
                # combine so its PE ops never wait on the vector engine
                (h, ib, pyT, ea, dsb_a, rec_a, rec_b) = prev
                r_sb = small_pool.tile([1, IB], F32R, tag="r_sb", name="r_sb")
                with nc.allow_low_precision("f32r is full-width fp32"):
                    nc.vector.scalar_tensor_tensor(
                        r_sb[:], dsb_a[:], grow_sb[0:1, 0:1], rec_b[:],
                        MULT, MULT)
                return r_sb

            def emit_combine(prev, r_sb):
                (h, ib, pyT, ea, dsb_a, rec_a, rec_b) = prev
                # broadcast r over the 10 adapter rows, pre-scale the
                # adapter probs, and fold the adapter output into the
                # attention accumulator (same PSUM bank)
                pea10 = psPo.tile([AT, IB], F32, tag="po", name="pea10")
                nc.tensor.matmul(pea10[:], ones1[:, :AT], r_sb[:],
                                 start=True, stop=True)
                ea_s = small_pool.tile([AT, IB], BF16, tag="ea_s",
                                       name="ea_s")
                with nc.allow_low_precision("bf16 probs"):
                    nc.vector.tensor_mul(ea_s[:], ea[:], pea10[:])
                nc.tensor.matmul(pyT[:], av_sb[:, h * HD:(h + 1) * HD],
                                 ea_s[:], start=False, stop=True,
                                 skip_group_check=True)
                # broadcast 1/den and apply in ONE [128,512] multiply
                b1 = psPo.tile([P, IB], F32, tag="po", name="b1")
                nc.tensor.matmul(b1[:], ones1[:], rec_a[:],
                                 start=True, stop=True)
                b1s = small_pool.tile([P, IB], F32, tag="b1s", name="b1s")
                nc.scalar.copy(b1s[:], b1[:])
                with nc.allow_low_precision("bf16 attention output"):
                    nc.vector.tensor_mul(yT_sb[:, h, bass.ts(ib, IB)],
                                         pyT[:], b1s[:])

            def emit_proj_chunk(gt, act_obs=4):
                # output projection for one 128-token tile (all heads);
                # act_obs of the 8 PSUM evacuations go to ACT, rest to DVE
                osb = out_pool.tile([P, C], F32, tag="osb", name="osb")
                for ob in range(C // 512):
                    po = psPo.tile([P, 512], F32, tag="po", name="po")
                    for hh in range(NH):
                        nc.tensor.matmul(
                            po[:],
                            yT_sb[:, hh, bass.ts(gt, P)],
                            wp_sb[:, hh, bass.ts(ob, 512)],
                            start=(hh == 0), stop=(hh == NH - 1))
                    if ob < act_obs:
                        nc.scalar.copy(osb[:, bass.ts(ob, 512)], po[:])
                    else:
                        nc.vector.tensor_copy(
                            out=osb[:, bass.ts(ob, 512)], in_=po[:])
                    if ob == 3:
                        nc.sync.dma_start(out_d[bass.ts(gt, P), :C // 2],
                                          osb[:, :C // 2])
                nc.sync.dma_start(out_d[bass.ts(gt, P), C // 2:],
                                  osb[:, C // 2:])

            proj_queue = []

            # block order (2,1,0,3): medium blocks first keep the PE
            # HAM-warm across the A->C transition; the dense ib=3 blocks and
            # their projection tiles form the tail, so the end of the kernel
            # is pure back-to-back matmul instead of overhead-dominated
            # small blocks
            IB_ORDER = (3, 2, 1, 0)
            for ibx, ib in enumerate(IB_ORDER):
                isl = bass.ts(ib, IB)
                nj = 4 * ib + 4
                if ibx >= 1:
                    prev_ib = IB_ORDER[ibx - 1]
                    proj_queue.extend(4 * prev_ib + tt for tt in range(4))
                for h in range(NH):
                    qT_h = qT_sb[:, h, :]

                    # adapter scores first: exp(ea) runs during the jt loop
                    pasT = psPo.tile([AT, IB], F32, tag="po", name="pasT")
                    nc.tensor.matmul(pasT[:], akT_sb[:, h, :], qT_h[:, isl],
                                     start=True, stop=True)
                    ea = small_pool.tile([AT, IB], BF16, tag="ea", name="ea")
                    nc.scalar.activation(ea[:], pasT[:], EXP, scale=ASCALE)

                    # emit r for the newest pending block (one block ahead
                    # of its combine)
                    if pending and pending[-1][1] is None:
                        pending[-1][1] = emit_r(pending[-1][0])

                    pyT = psY.tile([P, IB], F32, tag="py", name="pyT")
                    denp = psSm.tile([33, IB], F32, tag="sm", name="denp")

                    eTs = {}

                    def emit_scores(jt, h=h, ib=ib, isl=isl, qT_h=qT_h,
                                    eTs=eTs):
                        # diagonal tiles only see queries i >= 128*d within
                        # the block: compute the narrowed column range only
                        d = jt - 4 * ib
                        off = max(0, d) * P
                        qsl = slice(ib * IB + off, (ib + 1) * IB)
                        psT = psS.tile([P, IB], F32, tag="ps", name="psT")
                        nc.tensor.matmul(
                            psT[:, off:], kT_sb[:, h, bass.ts(jt, P)],
                            qT_h[:, qsl], start=True, stop=True)
                        eT = exp_pool.tile([P, IB], BF16, tag="eT",
                                           name="eT")
                        nc.scalar.activation(eT[:, off:], psT[:, off:], EXP,
                                             scale=SCALE)
                        if d >= 0:
                            # masks always on DVE: gpsimd tensor ops have
                            # multi-us fixed cost and sit on the eT->pv
                            # critical path
                            with nc.allow_low_precision("bf16 probs"):
                                nc.vector.tensor_mul(
                                    eT[:, off:], eT[:, off:],
                                    masks_sb[:, d, off:])
                        eTs[jt] = (eT, off)

                    emit_scores(0)
                    emit_scores(1)
                    emit_scores(2)
                    # adapter denominator at partition 32 of the den bank
                    nc.tensor.matmul(denp[32:33, :], ones128[:AT, :], ea[:],
                                     start=True, stop=True,
                                     tile_position=(0, 32))
                    for jt in range(nj):
                        if jt + 3 < nj:
                            emit_scores(jt + 3)
                        if jt == nj - 1 and pending:
                            prev, prev_r = pending.pop(0)
                            emit_combine(prev, prev_r)
                        eT, off = eTs.pop(jt)
                        nc.tensor.matmul(pyT[:, off:],
                                         v_sb[:, jt, h * HD:(h + 1) * HD],
                                         eT[:, off:], start=(jt == 0),
                                         stop=False, skip_group_check=True)
                        nc.tensor.matmul(denp[0:1, off:], ones128[:],
                                         eT[:, off:], start=(jt == 0),
                                         stop=(jt == nj - 1),
                                         skip_group_check=True)

                    # denominators -> SBUF, reshape onto all 128 lanes via
                    # tiny SBUF<->SBUF DMAs, reciprocal, reshape back.  The
                    # DMAs ride the scalar engine's HWDGE ring so they never
                    # queue behind the megabyte-sized wp/out transfers on
                    # the sync ring.
                    # one 33-partition copy moves BOTH denominators out of
                    # PSUM: DVE time is free-dim-serial, so this costs the
                    # same as a single [1,512] copy
                    dsb = small_pool.tile([33, IB], F32, tag="dsb",
                                          name="dsb")
                    nc.vector.tensor_copy(out=dsb[:], in_=denp[:])
                    dsb_a = dsb[0:1, :]
                    dsb_b = dsb[32:33, :]
                    rsh = small_pool.tile([P, 2, IB // P], F32, tag="rsh",
                                          name="rsh")
                    nc.scalar.dma_start(
                        rsh[:, 0, :], dsb_a.rearrange("x (p e) -> x p e", p=P))
                    nc.scalar.dma_start(
                        rsh[:, 1, :], dsb_b.rearrange("x (p e) -> x p e", p=P))
                    rrec = small_pool.tile([P, 2, IB // P], F32R, tag="rrec",
                                           name="rrec")
                    with nc.allow_low_precision(
                            "f32r is full-width fp32 storage"):
                        nc.vector.reciprocal(rrec[:], rsh[:])
                    rec_a = small_pool.tile([1, IB], F32R, tag="rec_a",
                                            name="rec_a")
                    nc.scalar.dma_start(
                        rec_a.rearrange("x (p e) -> x p e", p=P),
                        rrec[:, 0, :])
                    rec_b = small_pool.tile([1, IB], F32R, tag="rec_b",
                                            name="rec_b")
                    nc.scalar.dma_start(
                        rec_b.rearrange("x (p e) -> x p e", p=P),
                        rrec[:, 1, :])
                    pending.append([(h, ib, pyT, ea, dsb_a, rec_a, rec_b),
                                    None])

                    # proj tiles for i-block ib-1 become legal only once
                    # combine(ib-1, h3) has been emitted, i.e. during block
                    # (ib, h1) — so pop 1 chunk at h1/h2 and 2 at h3
                    if ib == IB_ORDER[-1]:
                        # keep one chunk in reserve: it provides PE cover
                        # for the final combine's reciprocal latency
                        npop = 0 if h == NH - 1 else 1
                    else:
                        npop = 1
                    for _ in range(npop):
                        if proj_queue:
                            # exp load on ACT scales with nj; shift the PSUM
                            # evacuations toward DVE in the dense blocks
                            emit_proj_chunk(proj_queue.pop(0),
                                            act_obs={4: 4, 8: 3, 12: 2, 16: 2}[nj])

            # tail: reserved chunk first (PE cover for the last combine's
            # reciprocal round trip), then the final combine + last tiles
            if proj_queue:
                emit_proj_chunk(proj_queue.pop(0))
            for prev, prev_r in pending:
                if prev_r is None:
                    prev_r = emit_r(prev)
                emit_combine(prev, prev_r)
            proj_queue.extend(4 * IB_ORDER[-1] + tt for tt in range(4))
            for gt in proj_queue:
                emit_proj_chunk(gt)

    nc.compile()
    return nc


LAST_RESULT = None

_ROPE_CACHE = None


def _rope_cos_sin_T():
    global _ROPE_CACHE
    if _ROPE_CACHE is None:
        bf = ml_dtypes.bfloat16
        theta = 1.0 / (10000.0 ** (np.arange(0, HD, 2, dtype=np.float32) / HD))
        idx = np.outer(np.arange(T, dtype=np.float32), theta)  # [T, 64]
        full = np.concatenate([idx, idx], axis=1)              # [T, 128]
        _ROPE_CACHE = (np.ascontiguousarray(np.cos(full).T).astype(bf),
                       np.ascontiguousarray(np.sin(full).T).astype(bf))
    return _ROPE_CACHE


def kernel(x, w_attn, w_proj, adapter_wte, gating):
    bf = ml_dtypes.bfloat16
    x = np.asarray(x, np.float32)
    w_attn = np.asarray(w_attn, np.float32)
    w_proj = np.asarray(w_proj, np.float32)
    adapter_wte = np.asarray(adapter_wte, np.float32)
    gating = np.asarray(gating, np.float32)

    def pmajor(a_cT, nrow):
        # [C, cols] -> [128, C//128, cols] partition-major contiguous
        return np.ascontiguousarray(
            a_cT.reshape(nrow, P, -1).transpose(1, 0, 2)).astype(bf)

    xT = pmajor(x[0].T, CT)                      # [128, 32, T]
    awteT = pmajor(adapter_wte.T, CT)            # [128, 32, 10]
    cosT, sinT = _rope_cos_sin_T()

    # rotate-half as a matmul: rot = R @ q (in [d, t] layout); pass R^T
    R = np.zeros((P, P), np.float32)
    for d in range(64):
        R[d, d + 64] = -1.0
        R[d + 64, d] = 1.0
    rotT = np.ascontiguousarray(R.T).astype(bf)

    # causal masks for the 4 diagonal alignments of a [128j, 512i] block
    pp = np.arange(P)[:, None]
    ff = np.arange(IB)[None, :]
    masks = np.stack([(pp + P * k <= ff) for k in range(NIB)],
                     axis=1).astype(bf)                     # [128, 4, 512]
    masks = np.ascontiguousarray(masks)

    grow = np.full((1, P), float(gating[0]), np.float32)

    nc = _build_nc()

    in_maps = []
    for m in range(NCORES):
        wq = w_attn[OW * m: OW * (m + 1)]
        wk = w_attn[C + OW * m: C + OW * (m + 1)]
        wv = w_attn[2 * C + OW * m: 2 * C + OW * (m + 1)]
        wqkT = pmajor(np.concatenate([wq, wk], axis=0).T, CT)
        wvT = pmajor(wv.T, CT)
        wpT = pmajor(w_proj[:, OW * m: OW * (m + 1)].T, NH)
        in_maps.append({
            "xT": xT, "wqkT": wqkT, "wvT": wvT, "wpT": wpT,
            "awteT": awteT, "cosT": cosT, "sinT": sinT, "rotT": rotT,
            "masks": masks, "grow": grow,
        })

    trace = bool(int(os.environ.get("BASS_KERNEL_TRACE", "0")))
    res = run_bass_kernel_spmd(nc, in_maps, core_ids=list(range(NCORES)),
                               trace=trace)
    global LAST_RESULT
    LAST_RESULT = res
    if trace:
        print("HW exec time:", res.exec_time_ns, "ns")
        print("trace:", res.instructions_and_trace[1]
              if res.instructions_and_trace else None)

    out = np.zeros((T, C), np.float64)
    for r in res.results:
        out += r["out"].astype(np.float64)
    return out.astype(np.float32)[None]
